# revision 4
# baseline (speedup 1.0000x reference)
"""FP4Net (bnb-FP4 quantize-dequantize 4-layer MLP) Trainium2 kernel.

Strategy (8 NeuronCores):
  - Data-parallel over batch for the matmuls: each core handles 1024 of 8192 rows.
  - FP4 quant-dequant of the weights is sharded 8x across cores (by output-row
    blocks, keeping the 64-elem FP4 blocks intact), computed exactly with fp32
    bit tricks on the vector engine, stored transposed (W.T layout) in fp16,
    then AllGathered (one AllGather per 128-row tile group = "quarter") so
    every core has all dequantized weights with fine-grained availability.
  - 4 chained fp16 matmul layers (fp32 PSUM accumulate); bias+ReLU/sigmoid
    epilogues on the scalar engine; activations resident in SBUF feature-major.
  - Queue discipline (avoids head-of-line blocking):
      vector (DVE): dequant arithmetic only.
      sync:   dequant input loads + dequant transposes + shard stores + y out.
      scalar: x load, bias loads, matmul weight-strip loads, epilogues.
      gpsimd: collectives ONLY (each AllGather blocks this queue until done).
      tensor: matmuls.
    Weight l+1's dequant+gather runs concurrently with layer l's matmuls.

Rounding trick: with g = 3*w/scale, the bnb FP4 codebook {0, 1/192, 1/6, 1/4,
1/3, 1/2, 2/3, 1} maps to {0, 1/64, 1/2, 3/4, 1, 3/2, 2, 3}: round-to-nearest
over that set == round g to 1 stored mantissa bit (round-half-up via exact
small-significand integer adds), clamped below at 1/2, plus a two-threshold
step for the {0, 1/64} region. Verified bit-exact vs the jax reference modulo
~1-ulp boundary fuzz (~1 flipped element per 16M weights on the actual data).
The round-half-up is fused into two tensor_scalar ops:
  te = ((ta & 0xFFE00000) + 0x00200000) & 0xFFC00000
which is exact because the add operands are multiples of 2^21 below 2^31
(10 significant bits, within the fp32-internal integer ALU's exact range).
"""
import sys
import numpy as np

for _p in ("/opt/trn_rl_repo", "/root/.axon_site/_ro/trn_rl_repo"):
    if _p not in sys.path:
        sys.path.append(_p)

N_CORES = 8
B, IN, H, OUT = 8192, 1024, 4096, 1024
BS = B // N_CORES          # batch shard per core
HS = H // N_CORES          # hidden-row shard per core (w1/w2/w3)
OS = OUT // N_CORES        # out-row shard per core (w4)

# FP4 codebook-derived threshold constants (g-space = 3*norm), f64 precision
_FP4_POS = np.array([0.0, 0.0052083333, 0.6666667, 1.0, 0.3333333, 0.5,
                     0.1666667, 0.25], dtype=np.float32)
_CS = np.sort(_FP4_POS).astype(np.float64)
_TL = float(np.float32(3.0 * (_CS[0] + _CS[1]) / 2.0))
_TH = float(np.float32(3.0 * (_CS[1] + _CS[2]) / 2.0))
LO_BITS = int(np.float32(1.0 / 64).view(np.uint32))   # 0x3C800000
BIG_BITS = 0x40400000                                  # bits of 3.0


def _i32(x):
    return int(np.uint32(x).view(np.int32))


_CACHED = {}

# weight dims per layer: (rows of W == dout, k == contraction)
WDIMS = {1: (H, IN), 2: (H, H), 3: (H, H), 4: (OUT, H)}
FDQ = 1024         # dequant chunk free-size (fp32 elems per partition)
NBQ = FDQ // 64    # fp4 blocks per chunk
DQ_PREF = 2        # dequant input prefetch depth (chunks)
STRIP_PREF = 1     # matmul weight strip prefetch depth (j-tiles)


def _build_nc():
    import concourse.bass as bass
    import concourse.mybir as mybir
    import concourse.tile as tile
    from concourse import bacc

    dt = mybir.dt
    Alu = mybir.AluOpType
    Act = mybir.ActivationFunctionType

    nc = bacc.Bacc("TRN2", target_bir_lowering=False, debug=False,
                   num_devices=N_CORES)

    # ---- I/O ----
    xs = nc.dram_tensor("xst", [IN, BS], dt.float16, kind="ExternalInput")
    w_in = {
        1: nc.dram_tensor("w1s", [HS, IN], dt.float32, kind="ExternalInput"),
        2: nc.dram_tensor("w2s", [HS, H], dt.float32, kind="ExternalInput"),
        3: nc.dram_tensor("w3s", [HS, H], dt.float32, kind="ExternalInput"),
        4: nc.dram_tensor("w4s", [OS, H], dt.float32, kind="ExternalInput"),
    }
    b_in = {
        1: nc.dram_tensor("b1", [128, H // 128], dt.float32, kind="ExternalInput"),
        2: nc.dram_tensor("b2", [128, H // 128], dt.float32, kind="ExternalInput"),
        3: nc.dram_tensor("b3", [128, H // 128], dt.float32, kind="ExternalInput"),
        4: nc.dram_tensor("b4", [128, OUT // 128], dt.float32, kind="ExternalInput"),
    }
    y_out = nc.dram_tensor("y", [OUT, BS], dt.float32, kind="ExternalOutput")

    # ---- internal DRAM: dequantized W.T-layout shard + per-r-tile gathers ----
    # dq_shard[l]: this core's [nrt, K, 128] f16 (row-tile r, k, out-in-tile)
    # dq_full[l][q]: gathered [N_CORES, K, 128] f16 for r-tile q of every core.
    dq_shard = {}
    dq_full = {}
    for l, (dout, k) in WDIMS.items():
        nrt = dout // N_CORES // 128
        dq_shard[l] = nc.dram_tensor(f"dqs{l}", [nrt, k, 128], dt.float16)
        dq_full[l] = [
            nc.dram_tensor(f"dqf{l}{q}", [N_CORES, k, 128], dt.float16,
                           addr_space="Shared")
            for q in range(nrt)]

    with tile.TileContext(nc) as tc:
        with (
            tc.tile_pool(name="const", bufs=1) as cpool,
            tc.tile_pool(name="bias", bufs=1) as bpool,
            tc.tile_pool(name="a0", bufs=1) as a0pool,
            tc.tile_pool(name="acts", bufs=2) as apool,
            tc.tile_pool(name="dqin", bufs=DQ_PREF) as dqin_pool,
            tc.tile_pool(name="dqtmp", bufs=1) as dqtmp_pool,
            tc.tile_pool(name="dqout", bufs=3) as dqout_pool,
            tc.tile_pool(name="dqtp", bufs=3) as dqtp_pool,
            tc.tile_pool(name="wt", bufs=2 * (STRIP_PREF + 1)) as wpool,
            tc.tile_pool(name="psum", bufs=8, space="PSUM") as pspool,
        ):
            # int32 constants for scalar_tensor_tensor scalars
            c_half = cpool.tile([128, 1], dt.int32)
            nc.vector.memset(c_half[:], _i32(0x3F000000))
            c_sign = cpool.tile([128, 1], dt.int32)
            nc.vector.memset(c_sign[:], _i32(0x80000000))

            # ---- biases -> SBUF [128, ntiles]; x -> a0 (scalar queue) ----
            b_sb = {}
            for l, (dout, _k) in WDIMS.items():
                nj = dout // 128
                bt = bpool.tile([128, nj], dt.float32, tag=f"bias{l}")
                nc.scalar.dma_start(bt[:], b_in[l][:])
                b_sb[l] = bt

            a_cur = a0pool.tile([128, IN // 128, BS], dt.float16)
            nc.scalar.dma_start(
                a_cur[:], xs[:].rearrange("(j p) b -> p j b", p=128))

            # ---- dequant job list: (layer, r-tile, chunk-in-rtile) ----
            dq_jobs = []
            for l, (dout, k) in WDIMS.items():
                nrt = dout // N_CORES // 128
                for r in range(nrt):
                    for cix in range(k // FDQ):
                        dq_jobs.append((l, r, cix))

            dqin_tiles = {}

            def emit_dqin(idx):
                l, r, cix = dq_jobs[idx]
                w = dqin_pool.tile([128, NBQ, 64], dt.float32, tag="dqw")
                nc.sync.dma_start(
                    w[:],
                    w_in[l][r * 128:(r + 1) * 128, cix * FDQ:(cix + 1) * FDQ]
                    .rearrange("p (b i) -> p b i", i=64))
                dqin_tiles[idx] = w

            def emit_dq_compute(idx):
                """Exact bnb-FP4 qdq of one [128 rows, FDQ k] chunk (DVE)."""
                l, r, cix = dq_jobs[idx]
                w = dqin_tiles.pop(idx)
                scale = dqtmp_pool.tile([128, NBQ, 1], dt.float32, tag="scale")
                nc.vector.tensor_reduce(scale[:], w[:], axis=mybir.AxisListType.X,
                                        op=Alu.max, apply_absolute_value=True)
                recip = dqtmp_pool.tile([128, NBQ, 1], dt.float32, tag="recip")
                nc.vector.reciprocal(recip[:], scale[:])
                s3 = dqtmp_pool.tile([128, NBQ, 1], dt.float32, tag="s3")
                nc.vector.tensor_scalar_mul(s3[:], scale[:], 1.0 / 3.0)
                g = dqtmp_pool.tile([128, NBQ, 64], dt.float32, tag="g")
                nc.vector.scalar_tensor_tensor(
                    g[:], w[:], 3.0, recip[:].broadcast_to((128, NBQ, 64)),
                    op0=Alu.mult, op1=Alu.mult)
                gi = g[:].bitcast(dt.int32)
                # NOTE: DVE ops must never write in-place onto their own input
                # (dual-port perf modes race), and int adds must keep few
                # significant bits (the int ALU path is fp32-internal).
                ta = dqtmp_pool.tile([128, NBQ, 64], dt.int32, tag="ta")
                nc.vector.tensor_scalar(ta[:], gi, _i32(0x7FFFFFFF), None,
                                        op0=Alu.bitwise_and)  # m0 = |g| bits
                tb = dqtmp_pool.tile([128, NBQ, 64], dt.int32, tag="tb")
                nc.vector.tensor_scalar(tb[:], ta[:], _i32(0xFFC00000), None,
                                        op0=Alu.bitwise_and)  # trunc
                tc_ = dqtmp_pool.tile([128, NBQ, 64], dt.int32, tag="tc")
                nc.vector.tensor_scalar(tc_[:], ta[:], _i32(0x00200000), _i32(1),
                                        op0=Alu.bitwise_and,
                                        op1=Alu.logical_shift_left)  # half<<1
                td = dqtmp_pool.tile([128, NBQ, 64], dt.int32, tag="td")
                nc.vector.tensor_tensor(td[:], tb[:], tc_[:],
                                        op=Alu.add)  # r2a (exact: 10+1 bits)
                af = ta[:].bitcast(dt.float32)  # |g| as float
                # M1L = (|g|>TL)*LO_BITS, M2B = (|g|>TH)*BIG_BITS -- the float
                # products are exact (consts have <=5 significant bits)
                nc.vector.tensor_scalar(tb[:], af, _TL, float(LO_BITS),
                                        op0=Alu.is_gt, op1=Alu.mult)
                nc.vector.tensor_scalar(tc_[:], af, _TH, float(BIG_BITS),
                                        op0=Alu.is_gt, op1=Alu.mult)
                sel = g[:].bitcast(dt.int32)  # g is dead; reuse as sel buffer
                nc.vector.tensor_tensor(sel, tb[:], tc_[:],
                                        op=Alu.add)  # sel (disjoint bits)
                nc.vector.scalar_tensor_tensor(
                    ta[:], td[:], c_half[:], sel,
                    op0=Alu.max, op1=Alu.min)  # mag (ta = |g| is dead)
                # sign comes from w (same sign as g since 3/scale > 0)
                nc.vector.scalar_tensor_tensor(
                    tb[:], w[:].bitcast(dt.int32), c_sign[:], ta[:],
                    op0=Alu.bitwise_and, op1=Alu.bitwise_or)  # signed
                dq = dqout_pool.tile([128, NBQ, 64], dt.float16, tag="dq")
                nc.vector.tensor_tensor(
                    dq[:], tb[:].bitcast(dt.float32),
                    s3[:].broadcast_to((128, NBQ, 64)), op=Alu.mult)
                # transpose to W.T layout and store this chunk to DRAM (sync)
                dqt = dqtp_pool.tile([128, FDQ // 128, 128], dt.float16,
                                     tag="dqt")
                nc.sync.dma_start_transpose(
                    dqt[:], dq[:].rearrange("p b i -> p (b i)"))
                nc.sync.dma_start(
                    dq_shard[l][r, cix * FDQ:(cix + 1) * FDQ, :]
                    .rearrange("(c p) h -> p c h", p=128),
                    dqt[:])

            def emit_allgather(l, q):
                nc.gpsimd.collective_compute(
                    "AllGather", Alu.bypass,
                    replica_groups=[list(range(N_CORES))],
                    ins=[dq_shard[l][q:q + 1]],
                    outs=[dq_full[l][q][:]],
                )

            # ---- emit the whole dequant + gather pipeline up front ----
            for i in range(min(DQ_PREF, len(dq_jobs))):
                emit_dqin(i)
            for i, (l, r, cix) in enumerate(dq_jobs):
                emit_dq_compute(i)
                if i + DQ_PREF < len(dq_jobs):
                    emit_dqin(i + DQ_PREF)
                if cix == WDIMS[l][1] // FDQ - 1:  # last chunk of this r-tile
                    emit_allgather(l, r)

            # ---- matmul layers: flat j-job list with cross-layer prefetch ----
            # job = (l, q, c): layer l, gathered r-tile q, source core c.
            # Full-layer j-tile index = c * nrt + q.
            mm_jobs = []
            for l, (dout, k) in WDIMS.items():
                nrt = dout // N_CORES // 128
                for q in range(nrt):
                    for c in range(N_CORES):
                        mm_jobs.append((l, q, c))

            strip_tiles = {}

            def emit_strip(idx):
                l, q, c = mm_jobs[idx]
                K = WDIMS[l][1]
                nk = K // 128
                half = nk // 2
                wts = []
                for i0 in (0, half):
                    wt_h = wpool.tile([128, 16, 128], dt.float16, tag="wt")
                    nc.scalar.dma_start(
                        wt_h[:, :half, :],
                        dq_full[l][q][c, i0 * 128:(i0 + half) * 128, :]
                        .rearrange("(i p) h -> p i h", p=128))
                    wts.append(wt_h)
                strip_tiles[idx] = wts

            a_next = None
            cur_l = 0
            for idx in range(min(STRIP_PREF, len(mm_jobs))):
                emit_strip(idx)
            for idx, (l, q, c) in enumerate(mm_jobs):
                if l != cur_l:
                    # new layer: previous layer's output becomes input
                    if a_next is not None:
                        a_cur = a_next
                    cur_l = l
                    dout, K = WDIMS[l]
                    nj = dout // 128
                    nrt = dout // N_CORES // 128
                    nk = K // 128
                    half = nk // 2
                    out_dt = dt.float32 if l == 4 else dt.float16
                    a_next = apool.tile([128, nj, BS], out_dt, tag="acts")
                j = c * nrt + q
                wts = strip_tiles.pop(idx)
                if idx + STRIP_PREF < len(mm_jobs):
                    emit_strip(idx + STRIP_PREF)
                ps = []
                for _n in range(BS // 512):
                    ps_t = pspool.tile([128, 512], dt.float32, tag="ps")
                    ps.append(ps_t)
                for i in range(nk):
                    for n in range(BS // 512):
                        nc.tensor.matmul(
                            ps[n][:], wts[i // half][:, i % half, :],
                            a_cur[:, i, n * 512:(n + 1) * 512],
                            start=(i == 0), stop=(i == nk - 1))
                for n in range(BS // 512):
                    if l == 4:
                        nc.scalar.activation(
                            a_next[:, j, n * 512:(n + 1) * 512], ps[n][:],
                            Act.Sigmoid, bias=b_sb[l][:, j:j + 1], scale=1.0)
                    else:
                        nc.scalar.activation(
                            a_next[:, j, n * 512:(n + 1) * 512], ps[n][:],
                            Act.Relu, bias=b_sb[l][:, j:j + 1], scale=1.0)

            # ---- output: feature-major [OUT, BS] (sync queue, now idle) ----
            for j in range(OUT // 128):
                nc.sync.dma_start(y_out[j * 128:(j + 1) * 128, :],
                                  a_next[:, j, :])

    nc.compile()
    return nc


def _get_nc():
    if "nc" not in _CACHED:
        _CACHED["nc"] = _build_nc()
    return _CACHED["nc"]


def kernel(**inputs):
    from concourse.bass_utils import run_bass_kernel_spmd

    x = np.asarray(inputs["x"], dtype=np.float32)
    ws = {l: np.ascontiguousarray(np.asarray(inputs[f"w{l}"], dtype=np.float32))
          for l in (1, 2, 3, 4)}
    bs = {l: np.ascontiguousarray(
        np.asarray(inputs[f"b{l}"], dtype=np.float32).reshape(-1, 128).T)
        for l in (1, 2, 3, 4)}

    nc = _get_nc()
    in_maps = []
    for c in range(N_CORES):
        m = {
            "xst": np.ascontiguousarray(
                x[c * BS:(c + 1) * BS].T.astype(np.float16)),
            "w1s": ws[1][c * HS:(c + 1) * HS],
            "w2s": ws[2][c * HS:(c + 1) * HS],
            "w3s": ws[3][c * HS:(c + 1) * HS],
            "w4s": ws[4][c * OS:(c + 1) * OS],
            "b1": bs[1], "b2": bs[2], "b3": bs[3], "b4": bs[4],
        }
        in_maps.append(m)

    res = run_bass_kernel_spmd(nc, in_maps, list(range(N_CORES)))
    out = np.empty((B, OUT), dtype=np.float32)
    for c in range(N_CORES):
        out[c * BS:(c + 1) * BS] = res.results[c]["y"].T
    return out


if __name__ == "__main__":
    rng = np.random.default_rng(0)
    ins = {
        "x": rng.standard_normal((B, IN)).astype(np.float32),
        "w1": (rng.standard_normal((H, IN)) * 0.1).astype(np.float32),
        "b1": np.zeros(H, np.float32),
        "w2": (rng.standard_normal((H, H)) * 0.1).astype(np.float32),
        "b2": np.zeros(H, np.float32),
        "w3": (rng.standard_normal((H, H)) * 0.1).astype(np.float32),
        "b3": np.zeros(H, np.float32),
        "w4": (rng.standard_normal((OUT, H)) * 0.1).astype(np.float32),
        "b4": np.zeros(OUT, np.float32),
    }
    y = kernel(**ins)
    print("kernel ran, output shape", y.shape, "mean", float(y.mean()))


# revision 9
# speedup vs baseline: 1.0217x; 1.0217x over previous
"""FP4Net (bnb-FP4 quantize-dequantize 4-layer MLP) Trainium2 kernel.

Strategy (8 NeuronCores):
  - Data-parallel over batch for the matmuls: each core handles 1024 of 8192 rows.
  - FP4 quant-dequant of the weights is sharded 8x across cores (by output-row
    blocks, keeping the 64-elem FP4 blocks intact), computed exactly with fp32
    bit tricks on the vector engine, stored transposed (W.T layout) in fp16,
    then AllGathered (one AllGather per 128-row tile group = "quarter") so
    every core has all dequantized weights with fine-grained availability.
  - 4 chained fp16 matmul layers (fp32 PSUM accumulate); bias+ReLU/sigmoid
    epilogues on the scalar engine; activations resident in SBUF feature-major.
  - Queue discipline (avoids head-of-line blocking):
      vector (DVE): dequant arithmetic only.
      sync:   dequant input loads + dequant transposes + shard stores + y out.
      scalar: x load, bias loads, matmul weight-strip loads, epilogues.
      gpsimd: collectives ONLY (each AllGather blocks this queue until done).
      tensor: matmuls.
    Weight l+1's dequant+gather runs concurrently with layer l's matmuls.

Rounding trick: with g = 3*w/scale, the bnb FP4 codebook {0, 1/192, 1/6, 1/4,
1/3, 1/2, 2/3, 1} maps to {0, 1/64, 1/2, 3/4, 1, 3/2, 2, 3}: round-to-nearest
over that set == round g to 1 stored mantissa bit (round-half-up via exact
small-significand integer adds), clamped below at 1/2, plus a two-threshold
step for the {0, 1/64} region. Verified bit-exact vs the jax reference modulo
~1-ulp boundary fuzz (~1 flipped element per 16M weights on the actual data).
The round-half-up is fused into two tensor_scalar ops:
  te = ((ta & 0xFFE00000) + 0x00200000) & 0xFFC00000
which is exact because the add operands are multiples of 2^21 below 2^31
(10 significant bits, within the fp32-internal integer ALU's exact range).
"""
import sys
import numpy as np

for _p in ("/opt/trn_rl_repo", "/root/.axon_site/_ro/trn_rl_repo"):
    if _p not in sys.path:
        sys.path.append(_p)

N_CORES = 8
B, IN, H, OUT = 8192, 1024, 4096, 1024
BS = B // N_CORES          # batch shard per core
HS = H // N_CORES          # hidden-row shard per core (w1/w2/w3)
OS = OUT // N_CORES        # out-row shard per core (w4)

# FP4 codebook-derived threshold constants (g-space = 3*norm), f64 precision
_FP4_POS = np.array([0.0, 0.0052083333, 0.6666667, 1.0, 0.3333333, 0.5,
                     0.1666667, 0.25], dtype=np.float32)
_CS = np.sort(_FP4_POS).astype(np.float64)
_TL = float(np.float32(3.0 * (_CS[0] + _CS[1]) / 2.0))
_TH = float(np.float32(3.0 * (_CS[1] + _CS[2]) / 2.0))
LO_BITS = int(np.float32(1.0 / 64).view(np.uint32))   # 0x3C800000
BIG_BITS = 0x40400000                                  # bits of 3.0


def _i32(x):
    return int(np.uint32(x).view(np.int32))


_CACHED = {}

# weight dims per layer: (rows of W == dout, k == contraction)
WDIMS = {1: (H, IN), 2: (H, H), 3: (H, H), 4: (OUT, H)}
FDQ = 1024         # dequant chunk free-size (fp32 elems per partition)
NBQ = FDQ // 64    # fp4 blocks per chunk
DQ_PREF = 2        # dequant input prefetch depth (chunks)
STRIP_PREF = 1     # matmul weight strip prefetch depth (j-tiles)


def _build_nc():
    import concourse.bass as bass
    import concourse.mybir as mybir
    import concourse.tile as tile
    from concourse import bacc

    dt = mybir.dt
    Alu = mybir.AluOpType
    Act = mybir.ActivationFunctionType

    nc = bacc.Bacc("TRN2", target_bir_lowering=False, debug=False,
                   num_devices=N_CORES)

    # ---- I/O ----
    xs = nc.dram_tensor("xst", [IN, BS], dt.float16, kind="ExternalInput")
    w_in = {
        1: nc.dram_tensor("w1s", [HS, IN], dt.float32, kind="ExternalInput"),
        2: nc.dram_tensor("w2s", [HS, H], dt.float32, kind="ExternalInput"),
        3: nc.dram_tensor("w3s", [HS, H], dt.float32, kind="ExternalInput"),
        4: nc.dram_tensor("w4s", [OS, H], dt.float32, kind="ExternalInput"),
    }
    b_in = {
        1: nc.dram_tensor("b1", [128, H // 128], dt.float32, kind="ExternalInput"),
        2: nc.dram_tensor("b2", [128, H // 128], dt.float32, kind="ExternalInput"),
        3: nc.dram_tensor("b3", [128, H // 128], dt.float32, kind="ExternalInput"),
        4: nc.dram_tensor("b4", [128, OUT // 128], dt.float32, kind="ExternalInput"),
    }
    y_out = nc.dram_tensor("y", [OUT, BS], dt.float32, kind="ExternalOutput")

    # ---- internal DRAM: dequantized W.T-layout shard + grouped gathers ----
    # dq_shard[l]: this core's [nrt, K, 128] f16 (row-tile r, k, out-in-tile)
    # dq_full[l][g]: gathered [N_CORES * AGG[l], K, 128] f16 for r-tile group g
    # of every core. AGG[l] r-tiles per AllGather: sized so each AG amortizes
    # the ~30us collective floor while still pipelining under the layer that
    # consumes it (w1 halves for a fast L1 start; w2 quarters to match L2's
    # consumption cadence; w3/w4 have slack).
    AGG = {1: 2, 2: 1, 3: 2, 4: 1}
    dq_shard = {}
    dq_full = {}
    for l, (dout, k) in WDIMS.items():
        nrt = dout // N_CORES // 128
        dq_shard[l] = nc.dram_tensor(f"dqs{l}", [nrt, k, 128], dt.float16)
        dq_full[l] = [
            nc.dram_tensor(f"dqf{l}{g}", [N_CORES * AGG[l], k, 128], dt.float16,
                           addr_space="Shared")
            for g in range(nrt // AGG[l])]

    with tile.TileContext(nc) as tc:
        with (
            tc.tile_pool(name="const", bufs=1) as cpool,
            tc.tile_pool(name="bias", bufs=1) as bpool,
            tc.tile_pool(name="a0", bufs=1) as a0pool,
            tc.tile_pool(name="acts", bufs=2) as apool,
            tc.tile_pool(name="dqin", bufs=DQ_PREF) as dqin_pool,
            tc.tile_pool(name="dqtmp", bufs=1) as dqtmp_pool,
            tc.tile_pool(name="dqout", bufs=3) as dqout_pool,
            tc.tile_pool(name="dqtp", bufs=3) as dqtp_pool,
            tc.tile_pool(name="wt", bufs=2 * (STRIP_PREF + 1)) as wpool,
            tc.tile_pool(name="psum", bufs=8, space="PSUM") as pspool,
        ):
            # int32 constants for scalar_tensor_tensor scalars
            c_half = cpool.tile([128, 1], dt.int32)
            nc.vector.memset(c_half[:], _i32(0x3F000000))
            c_sign = cpool.tile([128, 1], dt.int32)
            nc.vector.memset(c_sign[:], _i32(0x80000000))

            # ---- biases -> SBUF [128, ntiles]; x -> a0 (scalar queue) ----
            b_sb = {}
            for l, (dout, _k) in WDIMS.items():
                nj = dout // 128
                bt = bpool.tile([128, nj], dt.float32, tag=f"bias{l}")
                nc.scalar.dma_start(bt[:], b_in[l][:])
                b_sb[l] = bt

            a_cur = a0pool.tile([128, IN // 128, BS], dt.float16)
            nc.scalar.dma_start(
                a_cur[:], xs[:].rearrange("(j p) b -> p j b", p=128))

            # ---- dequant job list: (layer, r-tile, chunk-in-rtile) ----
            dq_jobs = []
            for l, (dout, k) in WDIMS.items():
                nrt = dout // N_CORES // 128
                for r in range(nrt):
                    for cix in range(k // FDQ):
                        dq_jobs.append((l, r, cix))

            dqin_tiles = {}

            def emit_dqin(idx):
                l, r, cix = dq_jobs[idx]
                w = dqin_pool.tile([128, NBQ, 64], dt.float32, tag="dqw")
                nc.sync.dma_start(
                    w[:],
                    w_in[l][r * 128:(r + 1) * 128, cix * FDQ:(cix + 1) * FDQ]
                    .rearrange("p (b i) -> p b i", i=64))
                dqin_tiles[idx] = w

            def emit_dq_compute(idx):
                """Exact bnb-FP4 qdq of one [128 rows, FDQ k] chunk (DVE)."""
                l, r, cix = dq_jobs[idx]
                w = dqin_tiles.pop(idx)
                scale = dqtmp_pool.tile([128, NBQ, 1], dt.float32, tag="scale")
                nc.vector.tensor_reduce(scale[:], w[:], axis=mybir.AxisListType.X,
                                        op=Alu.max, apply_absolute_value=True)
                recip = dqtmp_pool.tile([128, NBQ, 1], dt.float32, tag="recip")
                nc.vector.reciprocal(recip[:], scale[:])
                s3 = dqtmp_pool.tile([128, NBQ, 1], dt.float32, tag="s3")
                nc.vector.tensor_scalar_mul(s3[:], scale[:], 1.0 / 3.0)
                g = dqtmp_pool.tile([128, NBQ, 64], dt.float32, tag="g")
                nc.vector.scalar_tensor_tensor(
                    g[:], w[:], 3.0, recip[:].broadcast_to((128, NBQ, 64)),
                    op0=Alu.mult, op1=Alu.mult)
                gi = g[:].bitcast(dt.int32)
                # NOTE: DVE ops must never write in-place onto their own input
                # (dual-port perf modes race), and int adds must keep few
                # significant bits (the int ALU path is fp32-internal).
                ta = dqtmp_pool.tile([128, NBQ, 64], dt.int32, tag="ta")
                nc.vector.tensor_scalar(ta[:], gi, _i32(0x7FFFFFFF), None,
                                        op0=Alu.bitwise_and)  # m0 = |g| bits
                tb = dqtmp_pool.tile([128, NBQ, 64], dt.int32, tag="tb")
                nc.vector.tensor_scalar(tb[:], ta[:], _i32(0xFFC00000), None,
                                        op0=Alu.bitwise_and)  # trunc
                tc_ = dqtmp_pool.tile([128, NBQ, 64], dt.int32, tag="tc")
                nc.vector.tensor_scalar(tc_[:], ta[:], _i32(0x00200000), _i32(1),
                                        op0=Alu.bitwise_and,
                                        op1=Alu.logical_shift_left)  # half<<1
                td = dqtmp_pool.tile([128, NBQ, 64], dt.int32, tag="td")
                nc.vector.tensor_tensor(td[:], tb[:], tc_[:],
                                        op=Alu.add)  # r2a (exact: 10+1 bits)
                af = ta[:].bitcast(dt.float32)  # |g| as float
                # M1L = (|g|>TL)*LO_BITS, M2B = (|g|>TH)*BIG_BITS -- the float
                # products are exact (consts have <=5 significant bits)
                nc.vector.tensor_scalar(tb[:], af, _TL, float(LO_BITS),
                                        op0=Alu.is_gt, op1=Alu.mult)
                nc.vector.tensor_scalar(tc_[:], af, _TH, float(BIG_BITS),
                                        op0=Alu.is_gt, op1=Alu.mult)
                sel = g[:].bitcast(dt.int32)  # g is dead; reuse as sel buffer
                nc.vector.tensor_tensor(sel, tb[:], tc_[:],
                                        op=Alu.add)  # sel (disjoint bits)
                nc.vector.scalar_tensor_tensor(
                    ta[:], td[:], c_half[:], sel,
                    op0=Alu.max, op1=Alu.min)  # mag (ta = |g| is dead)
                # sign comes from w (same sign as g since 3/scale > 0)
                nc.vector.scalar_tensor_tensor(
                    tb[:], w[:].bitcast(dt.int32), c_sign[:], ta[:],
                    op0=Alu.bitwise_and, op1=Alu.bitwise_or)  # signed
                dq = dqout_pool.tile([128, NBQ, 64], dt.float16, tag="dq")
                nc.vector.tensor_tensor(
                    dq[:], tb[:].bitcast(dt.float32),
                    s3[:].broadcast_to((128, NBQ, 64)), op=Alu.mult)
                # transpose to W.T layout and store this chunk to DRAM (sync)
                dqt = dqtp_pool.tile([128, FDQ // 128, 128], dt.float16,
                                     tag="dqt")
                nc.sync.dma_start_transpose(
                    dqt[:], dq[:].rearrange("p b i -> p (b i)"))
                nc.sync.dma_start(
                    dq_shard[l][r, cix * FDQ:(cix + 1) * FDQ, :]
                    .rearrange("(c p) h -> p c h", p=128),
                    dqt[:])

            def emit_allgather(l, g):
                nc.gpsimd.collective_compute(
                    "AllGather", Alu.bypass,
                    replica_groups=[list(range(N_CORES))],
                    ins=[dq_shard[l][g * AGG[l]:(g + 1) * AGG[l]]],
                    outs=[dq_full[l][g][:]],
                )

            # ---- emit the whole dequant + gather pipeline up front ----
            for i in range(min(DQ_PREF, len(dq_jobs))):
                emit_dqin(i)
            for i, (l, r, cix) in enumerate(dq_jobs):
                emit_dq_compute(i)
                if i + DQ_PREF < len(dq_jobs):
                    emit_dqin(i + DQ_PREF)
                if (cix == WDIMS[l][1] // FDQ - 1
                        and r % AGG[l] == AGG[l] - 1):  # group's last chunk
                    emit_allgather(l, r // AGG[l])

            # ---- matmul layers: flat j-job list with cross-layer prefetch ----
            # job = (l, g, t): layer l, gather group g, index t within the
            # gathered tensor (t = source_core * AGG + r_within_group).
            # Full-layer j-tile index = (t // AGG) * nrt + g * AGG + t % AGG.
            mm_jobs = []
            for l, (dout, k) in WDIMS.items():
                nrt = dout // N_CORES // 128
                for g in range(nrt // AGG[l]):
                    for t in range(N_CORES * AGG[l]):
                        mm_jobs.append((l, g, t))

            strip_tiles = {}

            def emit_strip(idx):
                l, g, t = mm_jobs[idx]
                K = WDIMS[l][1]
                nk = K // 128
                half = nk // 2
                wts = []
                for i0 in (0, half):
                    wt_h = wpool.tile([128, 16, 128], dt.float16, tag="wt")
                    nc.scalar.dma_start(
                        wt_h[:, :half, :],
                        dq_full[l][g][t, i0 * 128:(i0 + half) * 128, :]
                        .rearrange("(i p) h -> p i h", p=128))
                    wts.append(wt_h)
                strip_tiles[idx] = wts

            a_next = None
            cur_l = 0
            for idx in range(min(STRIP_PREF, len(mm_jobs))):
                emit_strip(idx)
            for idx, (l, g, t) in enumerate(mm_jobs):
                if l != cur_l:
                    # new layer: previous layer's output becomes input
                    if a_next is not None:
                        a_cur = a_next
                    cur_l = l
                    dout, K = WDIMS[l]
                    nj = dout // 128
                    nrt = dout // N_CORES // 128
                    nk = K // 128
                    half = nk // 2
                    out_dt = dt.float32 if l == 4 else dt.float16
                    a_next = apool.tile([128, nj, BS], out_dt, tag="acts")
                j = (t // AGG[l]) * nrt + g * AGG[l] + t % AGG[l]
                wts = strip_tiles.pop(idx)
                if idx + STRIP_PREF < len(mm_jobs):
                    emit_strip(idx + STRIP_PREF)
                ps = []
                for _n in range(BS // 512):
                    ps_t = pspool.tile([128, 512], dt.float32, tag="ps")
                    ps.append(ps_t)
                for i in range(nk):
                    for n in range(BS // 512):
                        nc.tensor.matmul(
                            ps[n][:], wts[i // half][:, i % half, :],
                            a_cur[:, i, n * 512:(n + 1) * 512],
                            start=(i == 0), stop=(i == nk - 1))
                for n in range(BS // 512):
                    if l == 4:
                        nc.scalar.activation(
                            a_next[:, j, n * 512:(n + 1) * 512], ps[n][:],
                            Act.Sigmoid, bias=b_sb[l][:, j:j + 1], scale=1.0)
                    else:
                        nc.scalar.activation(
                            a_next[:, j, n * 512:(n + 1) * 512], ps[n][:],
                            Act.Relu, bias=b_sb[l][:, j:j + 1], scale=1.0)
                if l == 4:
                    # stream each output tile as soon as its epilogue is done
                    # (sync queue is idle by now)
                    nc.sync.dma_start(y_out[j * 128:(j + 1) * 128, :],
                                      a_next[:, j, :])

    nc.compile()
    return nc


def _get_nc():
    if "nc" not in _CACHED:
        _CACHED["nc"] = _build_nc()
    return _CACHED["nc"]


def kernel(**inputs):
    from concourse.bass_utils import run_bass_kernel_spmd

    x = np.asarray(inputs["x"], dtype=np.float32)
    ws = {l: np.ascontiguousarray(np.asarray(inputs[f"w{l}"], dtype=np.float32))
          for l in (1, 2, 3, 4)}
    bs = {l: np.ascontiguousarray(
        np.asarray(inputs[f"b{l}"], dtype=np.float32).reshape(-1, 128).T)
        for l in (1, 2, 3, 4)}

    nc = _get_nc()
    in_maps = []
    for c in range(N_CORES):
        m = {
            "xst": np.ascontiguousarray(
                x[c * BS:(c + 1) * BS].T.astype(np.float16)),
            "w1s": ws[1][c * HS:(c + 1) * HS],
            "w2s": ws[2][c * HS:(c + 1) * HS],
            "w3s": ws[3][c * HS:(c + 1) * HS],
            "w4s": ws[4][c * OS:(c + 1) * OS],
            "b1": bs[1], "b2": bs[2], "b3": bs[3], "b4": bs[4],
        }
        in_maps.append(m)

    res = run_bass_kernel_spmd(nc, in_maps, list(range(N_CORES)))
    out = np.empty((B, OUT), dtype=np.float32)
    for c in range(N_CORES):
        out[c * BS:(c + 1) * BS] = res.results[c]["y"].T
    return out


if __name__ == "__main__":
    rng = np.random.default_rng(0)
    ins = {
        "x": rng.standard_normal((B, IN)).astype(np.float32),
        "w1": (rng.standard_normal((H, IN)) * 0.1).astype(np.float32),
        "b1": np.zeros(H, np.float32),
        "w2": (rng.standard_normal((H, H)) * 0.1).astype(np.float32),
        "b2": np.zeros(H, np.float32),
        "w3": (rng.standard_normal((H, H)) * 0.1).astype(np.float32),
        "b3": np.zeros(H, np.float32),
        "w4": (rng.standard_normal((OUT, H)) * 0.1).astype(np.float32),
        "b4": np.zeros(OUT, np.float32),
    }
    y = kernel(**ins)
    print("kernel ran, output shape", y.shape, "mean", float(y.mean()))


# revision 13
# speedup vs baseline: 1.1948x; 1.1694x over previous
"""FP4Net (bnb-FP4 quantize-dequantize 4-layer MLP) Trainium2 kernel.

Strategy (8 NeuronCores):
  - Data-parallel over batch for the matmuls: each core handles 1024 of 8192 rows.
  - FP4 quant-dequant of the weights is sharded 8x across cores (by output-row
    blocks, keeping the 64-elem FP4 blocks intact), computed exactly with fp32
    bit tricks on the vector engine, stored transposed (W.T layout) in fp16,
    then AllGathered (one AllGather per 128-row tile group = "quarter") so
    every core has all dequantized weights with fine-grained availability.
  - 4 chained fp16 matmul layers (fp32 PSUM accumulate); bias+ReLU/sigmoid
    epilogues on the scalar engine; activations resident in SBUF feature-major.
  - Queue discipline (avoids head-of-line blocking):
      vector (DVE): dequant arithmetic only.
      sync:   dequant input loads + dequant transposes + shard stores + y out.
      scalar: x load, bias loads, matmul weight-strip loads, epilogues.
      gpsimd: collectives ONLY (each AllGather blocks this queue until done).
      tensor: matmuls.
    Weight l+1's dequant+gather runs concurrently with layer l's matmuls.

Rounding trick: with g = 3*w/scale, the bnb FP4 codebook {0, 1/192, 1/6, 1/4,
1/3, 1/2, 2/3, 1} maps to {0, 1/64, 1/2, 3/4, 1, 3/2, 2, 3}: round-to-nearest
over that set == round g to 1 stored mantissa bit (round-half-up via exact
small-significand integer adds), clamped below at 1/2, plus a two-threshold
step for the {0, 1/64} region. Verified bit-exact vs the jax reference modulo
~1-ulp boundary fuzz (~1 flipped element per 16M weights on the actual data).
The round-half-up is fused into two tensor_scalar ops:
  te = ((ta & 0xFFE00000) + 0x00200000) & 0xFFC00000
which is exact because the add operands are multiples of 2^21 below 2^31
(10 significant bits, within the fp32-internal integer ALU's exact range).
"""
import sys
import numpy as np

for _p in ("/opt/trn_rl_repo", "/root/.axon_site/_ro/trn_rl_repo"):
    if _p not in sys.path:
        sys.path.append(_p)

N_CORES = 8
B, IN, H, OUT = 8192, 1024, 4096, 1024
BS = B // N_CORES          # batch shard per core
HS = H // N_CORES          # hidden-row shard per core (w1/w2/w3)
OS = OUT // N_CORES        # out-row shard per core (w4)

# FP4 codebook-derived threshold constants (g-space = 3*norm), f64 precision
_FP4_POS = np.array([0.0, 0.0052083333, 0.6666667, 1.0, 0.3333333, 0.5,
                     0.1666667, 0.25], dtype=np.float32)
_CS = np.sort(_FP4_POS).astype(np.float64)
_TL = float(np.float32(3.0 * (_CS[0] + _CS[1]) / 2.0))
_TH = float(np.float32(3.0 * (_CS[1] + _CS[2]) / 2.0))
LO_BITS = int(np.float32(1.0 / 64).view(np.uint32))   # 0x3C800000
BIG_BITS = 0x40400000                                  # bits of 3.0


def _i32(x):
    return int(np.uint32(x).view(np.int32))


_CACHED = {}

# weight dims per layer: (rows of W == dout, k == contraction)
WDIMS = {1: (H, IN), 2: (H, H), 3: (H, H), 4: (OUT, H)}
FDQ = 1024         # dequant chunk free-size (fp32 elems per partition)
NBQ = FDQ // 64    # fp4 blocks per chunk
DQ_PREF = 2        # dequant input prefetch depth (chunks)
STRIP_PREF = 1     # matmul weight strip prefetch depth (j-tiles)


def _build_nc():
    import concourse.bass as bass
    import concourse.mybir as mybir
    import concourse.tile as tile
    from concourse import bacc

    dt = mybir.dt
    Alu = mybir.AluOpType
    Act = mybir.ActivationFunctionType

    nc = bacc.Bacc("TRN2", target_bir_lowering=False, debug=False,
                   num_devices=N_CORES)

    # ---- I/O ----
    xs = nc.dram_tensor("xst", [IN, BS], dt.float16, kind="ExternalInput")
    w_in = {
        1: nc.dram_tensor("w1s", [HS, IN], dt.float32, kind="ExternalInput"),
        2: nc.dram_tensor("w2s", [HS, H], dt.float32, kind="ExternalInput"),
        3: nc.dram_tensor("w3s", [HS, H], dt.float32, kind="ExternalInput"),
        4: nc.dram_tensor("w4s", [OS, H], dt.float32, kind="ExternalInput"),
    }
    b_in = {
        1: nc.dram_tensor("b1", [128, H // 128], dt.float32, kind="ExternalInput"),
        2: nc.dram_tensor("b2", [128, H // 128], dt.float32, kind="ExternalInput"),
        3: nc.dram_tensor("b3", [128, H // 128], dt.float32, kind="ExternalInput"),
        4: nc.dram_tensor("b4", [128, OUT // 128], dt.float32, kind="ExternalInput"),
    }
    y_out = nc.dram_tensor("y", [OUT, BS], dt.float32, kind="ExternalOutput")

    # ---- internal DRAM: dequantized W.T-layout shard + grouped gathers ----
    # dq_shard[l]: this core's [nrt, K, 128] f16 (row-tile r, k, out-in-tile)
    # dq_full[l][g]: gathered [N_CORES * AGG[l], K, 128] f16 for r-tile group g
    # of every core. AGG[l] r-tiles per AllGather: sized so each AG amortizes
    # the ~30us collective floor while still pipelining under the layer that
    # consumes it (w1 halves for a fast L1 start; w2 quarters to match L2's
    # consumption cadence; w3/w4 have slack).
    AGG = {1: 1, 2: 1, 3: 2, 4: 1}
    dq_shard = {}
    dq_full = {}
    for l, (dout, k) in WDIMS.items():
        nrt = dout // N_CORES // 128
        dq_shard[l] = nc.dram_tensor(f"dqs{l}", [nrt, k, 128], dt.float16)
        dq_full[l] = [
            nc.dram_tensor(f"dqf{l}{g}", [N_CORES * AGG[l], k, 128], dt.float16,
                           addr_space="Shared")
            for g in range(nrt // AGG[l])]

    with tile.TileContext(nc) as tc:
        with (
            tc.tile_pool(name="const", bufs=1) as cpool,
            tc.tile_pool(name="bias", bufs=1) as bpool,
            tc.tile_pool(name="a0", bufs=1) as a0pool,
            tc.tile_pool(name="acts", bufs=2) as apool,
            tc.tile_pool(name="dqin", bufs=DQ_PREF) as dqin_pool,
            tc.tile_pool(name="dqtmp", bufs=1) as dqtmp_pool,
            tc.tile_pool(name="dqout", bufs=3) as dqout_pool,
            tc.tile_pool(name="dqtp", bufs=2) as dqtp_pool,
            tc.tile_pool(name="wt", bufs=5) as wpool,
            tc.tile_pool(name="psum", bufs=8, space="PSUM") as pspool,
        ):
            # int32 constants for scalar_tensor_tensor scalars
            c_half = cpool.tile([128, 1], dt.int32)
            nc.vector.memset(c_half[:], _i32(0x3F000000))
            c_sign = cpool.tile([128, 1], dt.int32)
            nc.vector.memset(c_sign[:], _i32(0x80000000))

            # ---- biases -> SBUF [128, ntiles]; x -> a0 (scalar queue) ----
            b_sb = {}
            for l, (dout, _k) in WDIMS.items():
                nj = dout // 128
                bt = bpool.tile([128, nj], dt.float32, tag=f"bias{l}")
                nc.scalar.dma_start(bt[:], b_in[l][:])
                b_sb[l] = bt

            a_cur = a0pool.tile([128, IN // 128, BS], dt.float16)
            nc.scalar.dma_start(
                a_cur[:], xs[:].rearrange("(j p) b -> p j b", p=128))

            # ---- dequant job list: (layer, r-tile, chunk-in-rtile) ----
            dq_jobs = []
            for l, (dout, k) in WDIMS.items():
                nrt = dout // N_CORES // 128
                for r in range(nrt):
                    for cix in range(k // FDQ):
                        dq_jobs.append((l, r, cix))

            dqin_tiles = {}

            def emit_dqin(idx):
                l, r, cix = dq_jobs[idx]
                w = dqin_pool.tile([128, NBQ, 64], dt.float32, tag="dqw")
                nc.sync.dma_start(
                    w[:],
                    w_in[l][r * 128:(r + 1) * 128, cix * FDQ:(cix + 1) * FDQ]
                    .rearrange("p (b i) -> p b i", i=64))
                dqin_tiles[idx] = w

            def emit_dq_compute(idx):
                """Exact bnb-FP4 qdq of one [128 rows, FDQ k] chunk (DVE)."""
                l, r, cix = dq_jobs[idx]
                w = dqin_tiles.pop(idx)
                scale = dqtmp_pool.tile([128, NBQ, 1], dt.float32, tag="scale")
                nc.vector.tensor_reduce(scale[:], w[:], axis=mybir.AxisListType.X,
                                        op=Alu.max, apply_absolute_value=True)
                recip = dqtmp_pool.tile([128, NBQ, 1], dt.float32, tag="recip")
                nc.vector.reciprocal(recip[:], scale[:])
                s3 = dqtmp_pool.tile([128, NBQ, 1], dt.float32, tag="s3")
                nc.vector.tensor_scalar_mul(s3[:], scale[:], 1.0 / 3.0)
                g = dqtmp_pool.tile([128, NBQ, 64], dt.float32, tag="g")
                nc.vector.scalar_tensor_tensor(
                    g[:], w[:], 3.0, recip[:].broadcast_to((128, NBQ, 64)),
                    op0=Alu.mult, op1=Alu.mult)
                gi = g[:].bitcast(dt.int32)
                # NOTE: DVE ops must never write in-place onto their own input
                # (dual-port perf modes race), and int adds must keep few
                # significant bits (the int ALU path is fp32-internal).
                ta = dqtmp_pool.tile([128, NBQ, 64], dt.int32, tag="ta")
                nc.vector.tensor_scalar(ta[:], gi, _i32(0x7FFFFFFF), None,
                                        op0=Alu.bitwise_and)  # m0 = |g| bits
                tb = dqtmp_pool.tile([128, NBQ, 64], dt.int32, tag="tb")
                nc.vector.tensor_scalar(tb[:], ta[:], _i32(0xFFC00000), None,
                                        op0=Alu.bitwise_and)  # trunc
                tc_ = dqtmp_pool.tile([128, NBQ, 64], dt.int32, tag="tc")
                nc.vector.tensor_scalar(tc_[:], ta[:], _i32(0x00200000), _i32(1),
                                        op0=Alu.bitwise_and,
                                        op1=Alu.logical_shift_left)  # half<<1
                td = dqtmp_pool.tile([128, NBQ, 64], dt.int32, tag="td")
                nc.vector.tensor_tensor(td[:], tb[:], tc_[:],
                                        op=Alu.add)  # r2a (exact: 10+1 bits)
                af = ta[:].bitcast(dt.float32)  # |g| as float
                # M1L = (|g|>TL)*LO_BITS, M2B = (|g|>TH)*BIG_BITS -- the float
                # products are exact (consts have <=5 significant bits)
                nc.vector.tensor_scalar(tb[:], af, _TL, float(LO_BITS),
                                        op0=Alu.is_gt, op1=Alu.mult)
                nc.vector.tensor_scalar(tc_[:], af, _TH, float(BIG_BITS),
                                        op0=Alu.is_gt, op1=Alu.mult)
                sel = g[:].bitcast(dt.int32)  # g is dead; reuse as sel buffer
                nc.vector.tensor_tensor(sel, tb[:], tc_[:],
                                        op=Alu.add)  # sel (disjoint bits)
                nc.vector.scalar_tensor_tensor(
                    ta[:], td[:], c_half[:], sel,
                    op0=Alu.max, op1=Alu.min)  # mag (ta = |g| is dead)
                # sign comes from w (same sign as g since 3/scale > 0)
                nc.vector.scalar_tensor_tensor(
                    tb[:], w[:].bitcast(dt.int32), c_sign[:], ta[:],
                    op0=Alu.bitwise_and, op1=Alu.bitwise_or)  # signed
                dq = dqout_pool.tile([128, NBQ, 64], dt.float16, tag="dq")
                nc.vector.tensor_tensor(
                    dq[:], tb[:].bitcast(dt.float32),
                    s3[:].broadcast_to((128, NBQ, 64)), op=Alu.mult)
                # Transpose to W.T layout WITHOUT a DMA-transpose (Tile
                # serializes InstDmaTransposeAnt against collectives, which
                # would chain the dequant pipeline to the AllGather cadence).
                # Instead: DVE 32x32 block transpose + a block-permuting
                # plain store. tmp[32bo+u, 32fk+v] = dq[32bo+v, 32fk+u], so
                # writing tmp through the (fk u)(bo v)->(bo u)(fk v) view of
                # the [FDQ, 128] DRAM region lands element (k, o) correctly.
                dqt = dqtp_pool.tile([128, FDQ], dt.float16, tag="dqt")
                nc.vector.transpose(dqt[:], dq[:].rearrange("p b i -> p (b i)"))
                # One store per 32-partition group bo: the (u, fk, v) view of
                # the DRAM region pairs element-for-element with dqt's
                # [32, FDQ] slab (partition p = 32*bo + u, free f = 32*fk + v)
                for bo in range(4):
                    nc.sync.dma_start(
                        dq_shard[l][r, cix * FDQ:(cix + 1) * FDQ,
                                    bo * 32:(bo + 1) * 32]
                        .rearrange("(fk u) v -> u fk v", u=32),
                        dqt[bo * 32:(bo + 1) * 32, :])

            def emit_allgather(l, g):
                nc.gpsimd.collective_compute(
                    "AllGather", Alu.bypass,
                    replica_groups=[list(range(N_CORES))],
                    ins=[dq_shard[l][g * AGG[l]:(g + 1) * AGG[l]]],
                    outs=[dq_full[l][g][:]],
                )

            # ---- emit the whole dequant + gather pipeline up front ----
            for i in range(min(DQ_PREF, len(dq_jobs))):
                emit_dqin(i)
            for i, (l, r, cix) in enumerate(dq_jobs):
                emit_dq_compute(i)
                if i + DQ_PREF < len(dq_jobs):
                    emit_dqin(i + DQ_PREF)
                if (cix == WDIMS[l][1] // FDQ - 1
                        and r % AGG[l] == AGG[l] - 1):  # group's last chunk
                    emit_allgather(l, r // AGG[l])

            # ---- matmul layers: flat j-job list with cross-layer prefetch ----
            # job = (l, g, t): layer l, gather group g, index t within the
            # gathered tensor (t = source_core * AGG + r_within_group).
            # Full-layer j-tile index = (t // AGG) * nrt + g * AGG + t % AGG.
            mm_jobs = []
            for l, (dout, k) in WDIMS.items():
                nrt = dout // N_CORES // 128
                for g in range(nrt // AGG[l]):
                    for t in range(N_CORES * AGG[l]):
                        mm_jobs.append((l, g, t))

            strip_tiles = {}

            def emit_strip(idx):
                l, g, t = mm_jobs[idx]
                K = WDIMS[l][1]
                nk = K // 128
                half = nk // 2
                wts = []
                for i0 in (0, half):
                    wt_h = wpool.tile([128, 16, 128], dt.float16, tag="wt")
                    nc.scalar.dma_start(
                        wt_h[:, :half, :],
                        dq_full[l][g][t, i0 * 128:(i0 + half) * 128, :]
                        .rearrange("(i p) h -> p i h", p=128))
                    wts.append(wt_h)
                strip_tiles[idx] = wts

            a_next = None
            cur_l = 0
            for idx in range(min(STRIP_PREF, len(mm_jobs))):
                emit_strip(idx)
            for idx, (l, g, t) in enumerate(mm_jobs):
                if l != cur_l:
                    # new layer: previous layer's output becomes input
                    if a_next is not None:
                        a_cur = a_next
                    cur_l = l
                    dout, K = WDIMS[l]
                    nj = dout // 128
                    nrt = dout // N_CORES // 128
                    nk = K // 128
                    half = nk // 2
                    out_dt = dt.float32 if l == 4 else dt.float16
                    a_next = apool.tile([128, nj, BS], out_dt, tag="acts")
                j = (t // AGG[l]) * nrt + g * AGG[l] + t % AGG[l]
                wts = strip_tiles.pop(idx)
                if idx + STRIP_PREF < len(mm_jobs):
                    emit_strip(idx + STRIP_PREF)
                ps = []
                for _n in range(BS // 512):
                    ps_t = pspool.tile([128, 512], dt.float32, tag="ps")
                    ps.append(ps_t)
                for i in range(nk):
                    for n in range(BS // 512):
                        nc.tensor.matmul(
                            ps[n][:], wts[i // half][:, i % half, :],
                            a_cur[:, i, n * 512:(n + 1) * 512],
                            start=(i == 0), stop=(i == nk - 1))
                for n in range(BS // 512):
                    if l == 4:
                        nc.scalar.activation(
                            a_next[:, j, n * 512:(n + 1) * 512], ps[n][:],
                            Act.Sigmoid, bias=b_sb[l][:, j:j + 1], scale=1.0)
                    else:
                        nc.scalar.activation(
                            a_next[:, j, n * 512:(n + 1) * 512], ps[n][:],
                            Act.Relu, bias=b_sb[l][:, j:j + 1], scale=1.0)
                if l == 4:
                    # stream each output tile as soon as its epilogue is done
                    # (sync queue is idle by now)
                    nc.sync.dma_start(y_out[j * 128:(j + 1) * 128, :],
                                      a_next[:, j, :])

    nc.compile()
    return nc


def _get_nc():
    if "nc" not in _CACHED:
        _CACHED["nc"] = _build_nc()
    return _CACHED["nc"]


def kernel(**inputs):
    from concourse.bass_utils import run_bass_kernel_spmd

    x = np.asarray(inputs["x"], dtype=np.float32)
    ws = {l: np.ascontiguousarray(np.asarray(inputs[f"w{l}"], dtype=np.float32))
          for l in (1, 2, 3, 4)}
    bs = {l: np.ascontiguousarray(
        np.asarray(inputs[f"b{l}"], dtype=np.float32).reshape(-1, 128).T)
        for l in (1, 2, 3, 4)}

    nc = _get_nc()
    in_maps = []
    for c in range(N_CORES):
        m = {
            "xst": np.ascontiguousarray(
                x[c * BS:(c + 1) * BS].T.astype(np.float16)),
            "w1s": ws[1][c * HS:(c + 1) * HS],
            "w2s": ws[2][c * HS:(c + 1) * HS],
            "w3s": ws[3][c * HS:(c + 1) * HS],
            "w4s": ws[4][c * OS:(c + 1) * OS],
            "b1": bs[1], "b2": bs[2], "b3": bs[3], "b4": bs[4],
        }
        in_maps.append(m)

    res = run_bass_kernel_spmd(nc, in_maps, list(range(N_CORES)))
    out = np.empty((B, OUT), dtype=np.float32)
    for c in range(N_CORES):
        out[c * BS:(c + 1) * BS] = res.results[c]["y"].T
    return out


if __name__ == "__main__":
    rng = np.random.default_rng(0)
    ins = {
        "x": rng.standard_normal((B, IN)).astype(np.float32),
        "w1": (rng.standard_normal((H, IN)) * 0.1).astype(np.float32),
        "b1": np.zeros(H, np.float32),
        "w2": (rng.standard_normal((H, H)) * 0.1).astype(np.float32),
        "b2": np.zeros(H, np.float32),
        "w3": (rng.standard_normal((H, H)) * 0.1).astype(np.float32),
        "b3": np.zeros(H, np.float32),
        "w4": (rng.standard_normal((OUT, H)) * 0.1).astype(np.float32),
        "b4": np.zeros(OUT, np.float32),
    }
    y = kernel(**ins)
    print("kernel ran, output shape", y.shape, "mean", float(y.mean()))
